# revision 14
# baseline (speedup 1.0000x reference)
"""Trainium2 Bass kernel for column self-attention (nn_ColumnSelfAttention).

Reference computation (per column c, columns are independent attention
problems):
    q = (x @ Wq + bq) * head_dim**-0.5 ; k = x @ Wk + bk ; v = x @ Wv + bv
    scores[h,c,i,j] = sum_d q[i,c,h,d] k[j,c,h,d]
    scores = where(mask[j,c], scores, -1e4); p = softmax_j(scores)
    ctx[i,c,:] = concat_h(p @ v) ; out = ctx @ Wo + bo

Sharding: 256 columns split across 8 NeuronCores.  Columns are sorted by
unmasked-key count and dealt round-robin so all cores share one static
program: position p on every core gets capacity KCAP[p] = max count among
the 8 columns of that rank group.

Sparsity: masked keys contribute exactly 0 after softmax (exp(-1e4) == 0
in the reference), so K and V are only projected for the *unmasked* keys
of each column (packed on host, zero-padded to KCAP[p]).  That removes
the mask entirely: scores of padding rows are exp(0)=1 but their V rows
and denominator ("ones") entries are 0.  K^T is used directly as the
scores stationary; V is computed transposed (tokens on the free axis so
packing pays) then flipped per column with a PE-identity transpose.

Precision: the Q and K projections run in fp8(e4m3) with DoubleRow perf
mode (2 fp8 weights per PE cell -> 256-deep contraction per pass, ~1.4x
matmul throughput).  Host-side quantization scales (x*16, Wq*scale*256,
Wk*32) keep everything in e4m3's normal range; the combined 2^21 factor
is divided out for free by the exp activation's scale input.  V / PV /
Wo remain fp16 -- their error hits the output directly, while Q/K error
is damped by the softmax (measured end-to-end: 1.7e-2 vs the 2e-2 gate,
fp16-only is 5.3e-4).  Output is stored fp16 (halves the 12.6MB output
DMA; adds ~2e-4 error).  fp32 PSUM accumulation throughout; softmax is
exp on the scalar engine + reciprocal/mul on DVE, with the denominator
coming free from an appended ones-column on V.

A burst of dummy matmuls at t=0 (behind the first, tiny ident DMA on the
fast sync queue) warms the PE HAM clock-gate (1.2 -> 2.4 GHz) before the
first DMA-paced projection.
"""

import os
import numpy as np
import ml_dtypes

import concourse.bacc as bacc
import concourse.tile as tile
import concourse.mybir as mybir
from concourse import bass
from concourse.bass_utils import run_bass_kernel_spmd

R, C, E, H, D = 128, 256, 768, 12, 64
NCORES = 8
CLOC = C // NCORES            # 32 columns per core
BLK = 4                       # columns per block
NBLK = CLOC // BLK
T = BLK * R                   # 512 q-tokens per block
NTOK = CLOC * R               # 4096 q-tokens per core
NCH = E // 128                # 6 chunks of the embedding dim
NG = NCH // 2                 # 3 DoubleRow groups of 256
F16 = mybir.dt.float16
F32 = mybir.dt.float32
F8 = mybir.dt.float8e4
E4 = ml_dtypes.float8_e4m3
DR = mybir.MatmulPerfMode.DoubleRow
Act = mybir.ActivationFunctionType

XS = 16.0                     # x fp8 pre-scale
WSQ = 256.0                   # (Wq * head_dim**-0.5) fp8 pre-scale
WSK = 32.0                    # Wk fp8 pre-scale
EXPSCALE = 1.0 / (XS * XS * WSQ * WSK)   # 2^-21, folded into exp

LAST_RESULTS = None           # for test.py introspection


def build_program(with_bias: bool, kcap):
    """kcap: tuple of CLOC ints -- packed-key capacity per column position.

    with_bias=False (the benchmark case) uses the fp8 DoubleRow Q/K path;
    with_bias=True falls back to all-fp16 (bias folding under the fp8
    scale chain isn't worth the complexity)."""
    kcap = list(kcap)
    assert len(kcap) == CLOC and all(1 <= k <= 128 for k in kcap)
    fp8 = not with_bias
    off = [0]
    for k in kcap:
        off.append(off[-1] + k)
    tb = [off[(b + 1) * BLK] - off[b * BLK] for b in range(NBLK)]  # packed toks/blk
    TBMAX = max(tb)
    TB8 = (TBMAX + 15) // 16 * 16   # DoubleRow middle-dim stride must be %16

    nc = bacc.Bacc("TRN2", target_bir_lowering=False, debug=False)

    # x^T per core, host-pretransposed and block-major so every per-block
    # chunk DMA is fully contiguous.
    if fp8:
        # DoubleRow pair layout: x8[b, g, p, j, t] = e4m3(XS * x[b*T+t, (2g+j)*128+p])
        x8_d = nc.dram_tensor("x8", [NBLK, NG, 128, 2, T], F8,
                              kind="ExternalInput")
        xk8_d = nc.dram_tensor("xk8", [NBLK, NG, 128, 2, TB8], F8,
                               kind="ExternalInput")
        w8_d = {
            n: nc.dram_tensor(n, [NG, 128, 2, E], F8, kind="ExternalInput")
            for n in ("wq8", "wk8")
        }
    else:
        x_d = nc.dram_tensor("x", [NBLK, NCH, 128, T], F16,
                             kind="ExternalInput")
    xkv_d = nc.dram_tensor("xkv", [NBLK, NCH, 128, TBMAX], F16,
                           kind="ExternalInput")
    vones_d = nc.dram_tensor("vones", [128, CLOC], F16, kind="ExternalInput")
    ident_d = nc.dram_tensor("ident", [128, 128], F16, kind="ExternalInput")
    # fp16 weights host-prearranged per 128-row chunk
    w16_names = ("wv", "wo") if fp8 else ("wq", "wk", "wv", "wo")
    w_d = {
        n: nc.dram_tensor(n, [NCH, 128, E], F16, kind="ExternalInput")
        for n in w16_names
    }
    if with_bias:
        bqkv_d = {
            n: nc.dram_tensor(n, [128, NCH], F32, kind="ExternalInput")
            for n in ("bq", "bk", "bv")
        }
        bo_d = nc.dram_tensor("bo", [1, E], F16, kind="ExternalInput")
    o_d = nc.dram_tensor("o", [CLOC, R, E], F16, kind="ExternalOutput")

    with tile.TileContext(nc) as tc:
        with (
            tc.tile_pool(name="const", bufs=1) as const,
            tc.tile_pool(name="blk", bufs=2) as blkp,
            tc.tile_pool(name="col", bufs=4) as colp,
            tc.tile_pool(name="psmm", bufs=2, space="PSUM") as psmm,
            tc.tile_pool(name="pss", bufs=2, space="PSUM") as pssp,
            tc.tile_pool(name="pscx", bufs=2, space="PSUM") as pscx,
            tc.tile_pool(name="pstr", bufs=2, space="PSUM") as pstr,
        ):
            # ---- constants / weights
            w_sb = {
                n: [const.tile([128, E], F16, tag=f"{n}{k}", name=f"w_{n}{k}")
                    for k in range(NCH)]
                for n in w16_names
            }
            if fp8:
                w8_sb = {
                    n: [const.tile([128, 2, E], F8, tag=f"{n}{g}",
                                   name=f"w_{n}{g}")
                        for g in range(NG)]
                    for n in ("wq8", "wk8")
                }
            vones_sb = const.tile([128, CLOC], F16, tag="vones")
            ident_sb = const.tile([128, 128], F16, tag="ident")
            if fp8:
                x8t0 = [blkp.tile([128, 2, T], F8, tag=f"x8{g}",
                                  name=f"x8t0_{g}", bufs=3)
                        for g in range(NG)]
                xk8t0 = [blkp.tile([128, 2, TB8], F8, tag=f"xk8{g}",
                                   name=f"xk8t0_{g}", bufs=3)
                         for g in range(NG)]
            else:
                xt0 = [blkp.tile([128, T], F16, tag=f"xt{k}", name=f"xt0_{k}",
                                 bufs=3)
                       for k in range(NCH)]
            xk0 = [blkp.tile([128, TBMAX], F16, tag=f"xk{k}", name=f"xk0_{k}",
                             bufs=3)
                   for k in range(NCH)]
            qs = (nc.sync, nc.scalar, nc.gpsimd)
            # ident FIRST on the fast sync queue: it feeds the HAM warm-up
            # burst, which must start well before the bulk startup DMAs land
            nc.sync.dma_start(ident_sb[:], ident_d.ap())
            nc.gpsimd.dma_start(vones_sb[:], vones_d.ap())
            # stripe the Q-phase inputs first, then K, V, O phases in
            # consumption order so block-0 computes as chunks land
            if fp8:
                for g in range(NG):
                    qs[(2 * g) % 3].dma_start(x8t0[g][:], x8_d.ap()[0, g])
                    qs[(2 * g + 1) % 3].dma_start(
                        w8_sb["wq8"][g][:], w8_d["wq8"].ap()[g]
                    )
                for g in range(NG):
                    qs[(2 * g) % 3].dma_start(
                        xk8t0[g][:, :, 0:tb[0]],
                        xk8_d.ap()[0, g, :, :, 0:tb[0]],
                    )
                    qs[(2 * g + 1) % 3].dma_start(
                        w8_sb["wk8"][g][:], w8_d["wk8"].ap()[g]
                    )
            else:
                for k in range(NCH):
                    qs[(2 * k) % 3].dma_start(out=xt0[k][:], in_=x_d.ap()[0, k])
                    qs[(2 * k + 1) % 3].dma_start(
                        w_sb["wq"][k][:], w_d["wq"].ap()[k]
                    )
                for k in range(NCH):
                    qs[(2 * k) % 3].dma_start(
                        out=xk0[k][:, 0:tb[0]], in_=xkv_d.ap()[0, k, :, 0:tb[0]]
                    )
                    qs[(2 * k + 1) % 3].dma_start(
                        w_sb["wk"][k][:], w_d["wk"].ap()[k]
                    )
            if fp8:
                # fp16 packed keys still feed the V projection; they are
                # needed (t+~18us) before the wv weights finish, so issue
                # all six ahead of wv
                for k in range(NCH):
                    qs[k % 3].dma_start(
                        out=xk0[k][:, 0:tb[0]], in_=xkv_d.ap()[0, k, :, 0:tb[0]]
                    )
            for k in range(NCH):
                qs[(2 * k + 1) % 3].dma_start(w_sb["wv"][k][:], w_d["wv"].ap()[k])
            for k in range(NCH):
                qs[(2 * k + 1) % 3].dma_start(w_sb["wo"][k][:], w_d["wo"].ap()[k])
            if with_bias:
                bqkv_sb = {}
                for n in ("bq", "bk", "bv"):
                    bqkv_sb[n] = const.tile([128, NCH], F32, tag=n, name=f"b_{n}")
                    nc.gpsimd.dma_start(bqkv_sb[n][:], bqkv_d[n].ap())
                bo_sb = const.tile([1, E], F16, tag="bo")
                nc.gpsimd.dma_start(bo_sb[:], bo_d.ap())
                ones_sb = const.tile([1, 128], F16, tag="ones")
                nc.gpsimd.memset(ones_sb[:], 1.0)

            # ---- HAM warm-up: ~2.9us of back-to-back dummy matmuls right
            # after the (tiny, first-issued) ident DMA lands.  The PE clock
            # gate needs one busy 4096-cycle window to go 1.2 -> 2.4 GHz;
            # without this the whole DMA-paced block-0 projection runs cold
            # (the gate only flipped at ~18us into the baseline kernel).
            warm = psmm.tile([128, T], F32, tag="mm", name="warm")
            for _ in range(27):
                nc.tensor.matmul(
                    warm[:, 0:128], ident_sb[:], ident_sb[:],
                    start=True, stop=True,
                )

            pending_wo = None
            for b in range(NBLK):
                TB = tb[b]
                loff = [off[b * BLK + t] - off[b * BLK] for t in range(BLK)]

                if b == 0:
                    xk = xk0
                    if fp8:
                        x8t, xk8t = x8t0, xk8t0
                    else:
                        xt = xt0
                else:
                    # steady state: all input DMAs on the sync queue so the
                    # scalar queue is free for PSUM-drain copies (the psmm
                    # ring stalls projections if the drain lags)
                    xk = [blkp.tile([128, TBMAX], F16, tag=f"xk{k}",
                                    name=f"xk_{k}", bufs=3)
                          for k in range(NCH)]
                    if fp8:
                        x8t = [blkp.tile([128, 2, T], F8, tag=f"x8{g}",
                                         name=f"x8_{g}", bufs=3)
                               for g in range(NG)]
                        xk8t = [blkp.tile([128, 2, TB8], F8, tag=f"xk8{g}",
                                          name=f"xk8_{g}", bufs=3)
                                for g in range(NG)]
                        for g in range(NG):
                            nc.sync.dma_start(x8t[g][:], x8_d.ap()[b, g])
                        for g in range(NG):
                            nc.sync.dma_start(
                                xk8t[g][:, :, 0:TB],
                                xk8_d.ap()[b, g, :, :, 0:TB],
                            )
                    else:
                        xt = [blkp.tile([128, T], F16, tag=f"xt{k}",
                                        name=f"xt_{k}", bufs=3)
                              for k in range(NCH)]
                        for ec in range(NCH):
                            nc.sync.dma_start(out=xt[ec][:], in_=x_d.ap()[b, ec])
                    for ec in range(NCH):
                        # V-path keys ride the (lightly loaded) gpsimd queue
                        # so sync's x8/xk8 land without queueing behind them
                        nc.gpsimd.dma_start(
                            out=xk[ec][:, 0:TB],
                            in_=xkv_d.ap()[b, ec, :, 0:TB],
                        )

                # ---- Q^T over all tokens; K^T, V^T over packed keys ----
                qt = blkp.tile([128, NCH, T], F16, tag="qt")
                kt = blkp.tile([128, NCH, TBMAX], F16, tag="kt")
                vt = blkp.tile([128, NCH, TBMAX], F16, tag="vt")
                # zero-padded Q buffer (used below); memset early so the
                # gpsimd engine does it during the projection matmuls
                qz = blkp.tile([128, H, T], F16, tag="qz")
                if b < 2:
                    nc.gpsimd.memset(qz[:], 0.0)  # pool ring: zeros persist

                def drain_proj(dst, bname, co, ps, n_):
                    if with_bias:
                        nc.scalar.activation(
                            dst[:, co, 0:n_], ps[:, 0:n_], Act.Identity,
                            bias=bqkv_sb[bname][:, co : co + 1],
                        )
                    elif co % 2 == 0:   # alternate drain engines so the
                        nc.scalar.copy(dst[:, co, 0:n_], ps[:, 0:n_])
                    else:               # psum rings free promptly
                        nc.vector.tensor_copy(dst[:, co, 0:n_], ps[:, 0:n_])

                # projection emitters: (weight-list, pair-mode, src, n, dst)
                if fp8:
                    projs = (
                        (w8_sb["wq8"], True, x8t, T, qt, "bq"),
                        (w8_sb["wk8"], True, xk8t, TB, kt, "bk"),
                        (w_sb["wv"], False, xk, TB, vt, "bv"),
                    )
                else:
                    projs = (
                        (w_sb["wq"], False, xt, T, qt, "bq"),
                        (w_sb["wk"], False, xk, TB, kt, "bk"),
                        (w_sb["wv"], False, xk, TB, vt, "bv"),
                    )

                def proj_mm(ps, wtiles, pair, src_, n_, k, nk, co):
                    if pair:
                        nc.tensor.matmul(
                            ps[:, 0:n_],
                            wtiles[k][:, :, co * 128 : (co + 1) * 128],
                            src_[k][:, :, 0:n_],
                            start=(k == 0), stop=(k == nk - 1),
                            perf_mode=DR,
                        )
                    else:
                        nc.tensor.matmul(
                            ps[:, 0:n_],
                            wtiles[k][:, co * 128 : (co + 1) * 128],
                            src_[k][:, 0:n_],
                            start=(k == 0), stop=(k == nk - 1),
                        )

                for wtiles, pair, src_, n_, dst, bname in projs:
                    nk = NG if pair else NCH
                    if b == 0:
                        # k-outer with 6 concurrent psum groups: the first
                        # matmul only needs group 0 of the weights and x, so
                        # the PE starts as soon as the first chunks land.
                        pss6 = [
                            psmm.tile([128, T], F32, tag="mm", name="p6a"),
                            psmm.tile([128, T], F32, tag="mm", name="p6b"),
                            pssp.tile([128, 512], F32, tag="s", name="p6c"),
                            pssp.tile([128, 512], F32, tag="s", name="p6d"),
                            pscx.tile([128, 512], F32, tag="cx", name="p6e"),
                            pscx.tile([128, 512], F32, tag="cx", name="p6f"),
                        ]
                        for k in range(nk):
                            for co in range(NCH):
                                proj_mm(pss6[co], wtiles, pair, src_, n_,
                                        k, nk, co)
                        for co in range(NCH):
                            drain_proj(dst, bname, co, pss6[co], n_)
                    else:
                        for co in range(NCH):
                            ps = psmm.tile([128, T], F32, tag="mm", name="ps")
                            for k in range(nk):
                                proj_mm(ps, wtiles, pair, src_, n_, k, nk, co)
                            drain_proj(dst, bname, co, ps, n_)

                # ---- per-head zero-padded Q^T (base-partition-64 matmuls
                # into shared PSUM banks crash HW; contract K=128 instead,
                # with the other head's rows zeroed on the Q side.  Mixing
                # 64- and 128-row matmuls stalls the PE array ~100ns per
                # reconfig, so all heads use the K=128 form.)  Two fused 3D
                # copies: even heads sit at partitions 0-63 of their chunk,
                # odd heads at 64-127.
                qzv = qz.rearrange("p (c two) t -> p c two t", two=2)
                nc.vector.tensor_copy(qzv[0:64, :, 0, :], qt[0:64, :, :])
                nc.vector.tensor_copy(qzv[64:128, :, 1, :], qt[64:128, :, :])

                va = blkp.tile([128, BLK, H * 65], F16, tag="va")
                ets, pscs, ctxnts = {}, {}, {}

                def emit_vtrans(t):
                    # V natural per column: PE transpose of packed V^T,
                    # then DVE assembly into 65-stride head slots + ones.
                    p = b * BLK + t
                    kc = kcap[p]
                    lo = loff[t]
                    vtp = pstr.tile([128, NCH, 128], F16, tag="tr", name="vtp")
                    for ec in range(NCH):
                        nc.tensor.transpose(
                            vtp[0:kc, ec, :],
                            vt[:, ec, lo : lo + kc],
                            ident_sb[:],
                        )
                    dst = va[0:kc, t, :].rearrange(
                        "p (h x) -> p h x", x=65
                    )[:, :, 0:64]
                    src = vtp[0:kc, :, :].rearrange(
                        "p c (h d) -> p (c h) d", d=64
                    )
                    if with_bias:
                        # zero padding rows (v=bv there otherwise)
                        nc.vector.tensor_mul(
                            dst, src,
                            vones_sb[0:kc, p : p + 1]
                            .unsqueeze(2)
                            .broadcast_to((kc, H, 64)),
                        )
                    else:
                        nc.vector.tensor_copy(dst, src)
                    ones_dst = va[0:kc, t, :].rearrange(
                        "p (h x) -> p h x", x=65
                    )[:, :, 64:65]
                    nc.vector.tensor_copy(
                        ones_dst,
                        vones_sb[0:kc, p : p + 1].unsqueeze(2).broadcast_to(
                            (kc, H, 1)
                        ),
                    )

                def emit_scores(t):
                    p = b * BLK + t
                    kc = kcap[p]
                    lo = loff[t]
                    et = colp.tile([128, H * 128], F16, tag="et",
                                   name=f"et_{b}_{t}")
                    for g3 in range(3):
                        pss = pssp.tile([128, 512], F32, tag="s", name="pss")
                        for hh in range(4):
                            h = g3 * 4 + hh
                            nc.tensor.matmul(
                                pss[0:kc, hh * 128 : (hh + 1) * 128],
                                kt[:, h // 2, lo : lo + kc],
                                qz[:, h, t * 128 : (t + 1) * 128],
                                start=(hh == 0),
                                stop=(hh == 3),
                            )
                        # fp8 path: q,k carry the 2^21 quantization scale;
                        # exp divides it back out for free
                        nc.scalar.activation(
                            et[0:kc, g3 * 512 : (g3 + 1) * 512],
                            pss[0:kc, :], Act.Exp,
                            scale=EXPSCALE if fp8 else 1.0,
                        )
                    ets[t] = et

                def emit_pv(t):
                    p = b * BLK + t
                    kc = kcap[p]
                    et = ets[t]
                    psc = []
                    for g2 in range(2):
                        pc = pscx.tile([128, 390], F32, tag="cx", name="pc")
                        for hh in range(6):
                            h = g2 * 6 + hh
                            nc.tensor.matmul(
                                pc[:, hh * 65 : (hh + 1) * 65],
                                et[0:kc, h * 128 : (h + 1) * 128],
                                va[0:kc, t, h * 65 : (h + 1) * 65],
                                start=(hh == 0),
                                stop=(hh == 5),
                            )
                        psc.append(pc)
                    pscs[t] = psc

                def emit_norm_tr(t):
                    psc = pscs.pop(t)
                    recip = colp.tile([128, H], F32, tag="recip", name="recip")
                    ctxn = colp.tile([128, E], F16, tag="ctxn", name="ctxn")
                    for g2 in range(2):
                        grp = psc[g2].rearrange("p (h x) -> p h x", x=65)
                        nc.vector.reciprocal(
                            recip[:, g2 * 6 : (g2 + 1) * 6].unsqueeze(2),
                            grp[:, :, 64:65],
                        )
                        nc.vector.tensor_mul(
                            ctxn[:, g2 * 384 : (g2 + 1) * 384].rearrange(
                                "p (h d) -> p h d", d=64
                            ),
                            grp[:, :, 0:64],
                            recip[:, g2 * 6 : (g2 + 1) * 6]
                            .unsqueeze(2)
                            .broadcast_to((128, 6, 64)),
                        )
                    pst = pstr.tile([128, NCH, 128], F16, tag="tr", name="pst")
                    for ec in range(NCH):
                        nc.tensor.transpose(
                            pst[:, ec, :],
                            ctxn[:, ec * 128 : (ec + 1) * 128],
                            ident_sb[:],
                        )
                    ctxnt = colp.tile([128, NCH, 128], F16, tag="ctxnt",
                                      name="ctxnt")
                    nc.vector.tensor_copy(ctxnt[:], pst[:])
                    ctxnts[t] = ctxnt

                def emit_wo(t, cg, store, cross_block=False, split_dma=False):
                    if t not in store:
                        return
                    ctxnt = store.pop(t)
                    osb = colp.tile([128, E], F16, tag="osb", name="osb")
                    for half in range(2):
                        po = psmm.tile([128, T], F32, tag="mm", name="po")
                        if with_bias:
                            nc.tensor.matmul(
                                po[:, 0:384], ones_sb[:],
                                bo_sb[:, half * 384 : (half + 1) * 384],
                                start=True, stop=False,
                            )
                        for k in range(NCH):
                            nc.tensor.matmul(
                                po[:, 0:384],
                                ctxnt[:, k, :],
                                w_sb["wo"][k][:, half * 384 : (half + 1) * 384],
                                start=(k == 0 and not with_bias),
                                stop=(k == NCH - 1),
                            )
                        if cross_block:  # vector is congested at block start
                            nc.scalar.copy(
                                osb[:, half * 384 : (half + 1) * 384],
                                po[:, 0:384],
                            )
                        else:
                            nc.vector.tensor_copy(
                                osb[:, half * 384 : (half + 1) * 384],
                                po[:, 0:384],
                            )
                    # one fully-contiguous DRAM write per column (a half-E
                    # split makes 768B strided lines -- ~11GB/s, 3x slower).
                    # The last two columns take the two fast HWDGE queues
                    # (input traffic is over by then); the rest ride gpsimd.
                    if cg == CLOC - 1:
                        oeng = nc.scalar
                    elif cg == CLOC - 2:
                        oeng = nc.sync
                    else:
                        oeng = nc.gpsimd
                    oeng.dma_start(o_d.ap()[cg], osb[:])

                last_blk = b == NBLK - 1
                emit_scores(0)
                emit_vtrans(0)
                emit_scores(1)
                for t in range(BLK):
                    emit_pv(t)
                    if t + 1 < BLK:
                        emit_vtrans(t + 1)
                    if t >= 1:
                        emit_wo(t - 1, b * BLK + t - 1, ctxnts,
                                split_dma=(last_blk and t == BLK - 1))
                    elif pending_wo is not None:
                        pending_wo()           # last column of previous block
                        pending_wo = None
                    if t + 2 < BLK:
                        emit_scores(t + 2)
                    emit_norm_tr(t)
                if last_blk:
                    # no next block to hide it in: flush the last column now,
                    # halves striped across both output queues
                    emit_wo(BLK - 1, b * BLK + BLK - 1, ctxnts,
                            split_dma=True)
                else:
                    import functools
                    pending_wo = functools.partial(
                        emit_wo, BLK - 1, b * BLK + BLK - 1, ctxnts,
                        cross_block=True,
                    )
    nc.compile()
    return nc


_PROGRAMS = {}


def _get_program(with_bias: bool, kcap: tuple):
    key = (with_bias, kcap)
    if key not in _PROGRAMS:
        _PROGRAMS[key] = build_program(with_bias, kcap)
    return _PROGRAMS[key]


def plan_columns(mask):
    """mask: (R, C) bool, True = valid key.  Returns per-core column lists,
    per-position capacities, and per-(core,position) key indices."""
    counts = mask.sum(axis=0)                       # (C,)
    order = np.argsort(-counts, kind="stable")
    cols = [[int(order[8 * p + i]) for p in range(CLOC)] for i in range(NCORES)]
    kcap = tuple(
        int(counts[order[8 * p : 8 * p + 8]].max()) for p in range(CLOC)
    )
    return cols, kcap


def make_in_maps(x, self_attn_padding_mask, Wq, bq, Wk, bk, Wv, bv, Wo, bo,
                 with_bias, cols, kcap):
    scaling = float(D) ** -0.5
    fp8 = not with_bias
    def prep_w(W, s=1.0):
        w = (np.asarray(W, np.float32) * s).astype(np.float16)
        return np.ascontiguousarray(w.reshape(NCH, 128, E))
    def prep_w8(W, s):
        # [NG, 128, 2, E]: row (2g+j)*128+p at [g, p, j]
        w = (np.asarray(W, np.float32) * s).astype(E4)
        return np.ascontiguousarray(
            w.reshape(NG, 2, 128, E).transpose(0, 2, 1, 3)
        )
    wv, wo = prep_w(Wv), prep_w(Wo)
    if fp8:
        wq8 = prep_w8(Wq, scaling * WSQ)
        wk8 = prep_w8(Wk, WSK)
    else:
        wq, wk = prep_w(Wq, scaling), prep_w(Wk)
    mask = np.asarray(self_attn_padding_mask)[0]                   # (R, C)
    x32 = np.asarray(x, np.float32)[:, :, 0, :]                    # (R, C, E)
    xf = x32.astype(np.float16)
    ident = np.eye(128, dtype=np.float16)
    off = np.concatenate([[0], np.cumsum(kcap)]).astype(int)
    tb = [int(off[(b + 1) * BLK] - off[b * BLK]) for b in range(NBLK)]
    TBMAX = max(tb)
    TB8 = (TBMAX + 15) // 16 * 16
    in_maps = []
    for i in range(NCORES):
        ci = cols[i]
        # packed keys (fp16 for V, fp8 for K), per-block padded
        xps = np.zeros((NBLK, NCH, 128, TBMAX), np.float16)
        xp32b = np.zeros((NBLK, TB8, E), np.float32)
        vo = np.zeros((128, CLOC), np.float16)
        for b in range(NBLK):
            xp = np.zeros((TBMAX, E), np.float16)
            for t in range(BLK):
                p = b * BLK + t
                col = ci[p]
                idx = np.nonzero(mask[:, col])[0]
                lo = int(off[p] - off[b * BLK])
                xp[lo : lo + len(idx)] = xf[idx, col]
                xp32b[b, lo : lo + len(idx)] = x32[idx, col]
                vo[: len(idx), p] = 1.0
            xps[b] = xp.reshape(TBMAX, NCH, 128).transpose(1, 2, 0)
        m = {
            "xkv": xps,
            "vones": vo,
            "wv": wv, "wo": wo,
            "ident": ident,
        }
        if fp8:
            # full x^T in DoubleRow pair layout [NBLK, NG, 128, 2, T]
            x8 = (
                (x32[:, ci] * XS).astype(E4)       # (R, CLOC, E)
                .transpose(1, 0, 2)                # (CLOC, R, E)
                .reshape(NBLK, T, NG, 2, 128)
                .transpose(0, 2, 4, 3, 1)          # (NBLK, NG, 128, 2, T)
            )
            xk8 = (
                (xp32b * XS).astype(E4)            # (NBLK, TB8, E)
                .reshape(NBLK, TB8, NG, 2, 128)
                .transpose(0, 2, 4, 3, 1)          # (NBLK, NG, 128, 2, TB8)
            )
            m["x8"] = np.ascontiguousarray(x8)
            m["xk8"] = np.ascontiguousarray(xk8)
            m["wq8"] = wq8
            m["wk8"] = wk8
        else:
            xs = (
                xf[:, ci]                          # (R, CLOC, E)
                .transpose(1, 0, 2)                # (CLOC, R, E)
                .reshape(NBLK, T, NCH, 128)
                .transpose(0, 2, 3, 1)             # (NBLK, NCH, 128, T)
            )
            m["x"] = np.ascontiguousarray(xs)
            m["wq"] = wq
            m["wk"] = wk
        if with_bias:
            m["bq"] = np.ascontiguousarray(
                (np.asarray(bq, np.float32) * scaling).reshape(NCH, 128).T
            )
            m["bk"] = np.ascontiguousarray(
                np.asarray(bk, np.float32).reshape(NCH, 128).T
            )
            m["bv"] = np.ascontiguousarray(
                np.asarray(bv, np.float32).reshape(NCH, 128).T
            )
            m["bo"] = np.asarray(bo, np.float32).astype(np.float16).reshape(1, E)
        in_maps.append(m)
    return in_maps


def assemble_output(shards, cols):
    out = np.empty((R, C, 1, E), np.float32)
    for i in range(NCORES):
        out[:, cols[i], 0, :] = np.asarray(shards[i], np.float32).transpose(1, 0, 2)
    return out


def kernel(x, self_attn_padding_mask, Wq, bq, Wk, bk, Wv, bv, Wo, bo):
    global LAST_RESULTS
    with_bias = any(
        bool(np.any(np.asarray(b))) for b in (bq, bk, bv, bo)
    )
    mask = np.asarray(self_attn_padding_mask)[0]
    cols, kcap = plan_columns(mask)
    nc = _get_program(with_bias, kcap)
    in_maps = make_in_maps(
        x, self_attn_padding_mask, Wq, bq, Wk, bk, Wv, bv, Wo, bo, with_bias,
        cols, kcap,
    )
    trace = os.environ.get("KERNEL_TRACE", "") not in ("", "0")
    res = run_bass_kernel_spmd(
        nc, in_maps, core_ids=list(range(NCORES)), trace=trace
    )
    LAST_RESULTS = res
    return assemble_output([res.results[i]["o"] for i in range(NCORES)], cols)


# revision 19
# speedup vs baseline: 1.0919x; 1.0919x over previous
"""Trainium2 Bass kernel for column self-attention (nn_ColumnSelfAttention).

Reference computation (per column c, columns are independent attention
problems):
    q = (x @ Wq + bq) * head_dim**-0.5 ; k = x @ Wk + bk ; v = x @ Wv + bv
    scores[h,c,i,j] = sum_d q[i,c,h,d] k[j,c,h,d]
    scores = where(mask[j,c], scores, -1e4); p = softmax_j(scores)
    ctx[i,c,:] = concat_h(p @ v) ; out = ctx @ Wo + bo

Sharding: 256 columns split across 8 NeuronCores.  Columns are sorted by
unmasked-key count and dealt round-robin so all cores share one static
program: position p on every core gets capacity KCAP[p] = max count among
the 8 columns of that rank group.

Sparsity: masked keys contribute exactly 0 after softmax (exp(-1e4) == 0
in the reference), so K and V are only projected for the *unmasked* keys
of each column (packed on host, zero-padded to KCAP[p]).  That removes
the mask entirely: scores of padding rows are exp(0)=1 but their V rows
and denominator ("ones") entries are 0.  K^T is used directly as the
scores stationary; V is computed transposed (tokens on the free axis so
packing pays) then flipped per column with a PE-identity transpose.

Precision: the Q and K projections run in fp8(e4m3) with DoubleRow perf
mode (2 fp8 weights per PE cell -> 256-deep contraction per pass, ~1.4x
matmul throughput).  Host-side quantization scales (x*16, Wq*scale*256,
Wk*32) keep everything in e4m3's normal range; the combined 2^21 factor
is divided out for free by the exp activation's scale input.  V / PV /
Wo remain fp16 -- their error hits the output directly, while Q/K error
is damped by the softmax (measured end-to-end: 1.7e-2 vs the 2e-2 gate,
fp16-only is 5.3e-4).  Output is stored fp16 (halves the 12.6MB output
DMA; adds ~2e-4 error).  fp32 PSUM accumulation throughout; softmax is
exp on the scalar engine + reciprocal/mul on DVE, with the denominator
coming free from an appended ones-column on V.

A burst of dummy matmuls at t=0 (behind the first, tiny ident DMA on the
fast sync queue) warms the PE HAM clock-gate (1.2 -> 2.4 GHz) before the
first DMA-paced projection.
"""

import os
import numpy as np
import ml_dtypes

import concourse.bacc as bacc
import concourse.tile as tile
import concourse.mybir as mybir
from concourse import bass
from concourse.bass_utils import run_bass_kernel_spmd

R, C, E, H, D = 128, 256, 768, 12, 64
NCORES = 8
CLOC = C // NCORES            # 32 columns per core
BLK = 4                       # columns per block
NBLK = CLOC // BLK
T = BLK * R                   # 512 q-tokens per block
NTOK = CLOC * R               # 4096 q-tokens per core
NCH = E // 128                # 6 chunks of the embedding dim
NG = NCH // 2                 # 3 DoubleRow groups of 256
F16 = mybir.dt.float16
F32 = mybir.dt.float32
F8 = mybir.dt.float8e4
E4 = ml_dtypes.float8_e4m3
DR = mybir.MatmulPerfMode.DoubleRow
Act = mybir.ActivationFunctionType

XS = 16.0                     # x fp8 pre-scale
WSQ = 256.0                   # (Wq * head_dim**-0.5) fp8 pre-scale
WSK = 32.0                    # Wk fp8 pre-scale
EXPSCALE = 1.0 / (XS * XS * WSQ * WSK)   # 2^-21, folded into exp

LAST_RESULTS = None           # for test.py introspection


def build_program(with_bias: bool, kcap):
    """kcap: tuple of CLOC ints -- packed-key capacity per column position.

    with_bias=False (the benchmark case) uses the fp8 DoubleRow Q/K path;
    with_bias=True falls back to all-fp16 (bias folding under the fp8
    scale chain isn't worth the complexity)."""
    kcap = list(kcap)
    assert len(kcap) == CLOC and all(1 <= k <= 128 for k in kcap)
    fp8 = not with_bias
    off = [0]
    for k in kcap:
        off.append(off[-1] + k)
    tb = [off[(b + 1) * BLK] - off[b * BLK] for b in range(NBLK)]  # packed toks/blk
    TBMAX = max(tb)
    TB8 = (TBMAX + 15) // 16 * 16   # DoubleRow middle-dim stride must be %16

    nc = bacc.Bacc("TRN2", target_bir_lowering=False, debug=False)

    # x^T per core, host-pretransposed and block-major so every per-block
    # chunk DMA is fully contiguous.
    if fp8:
        # DoubleRow pair layout: x8[b, g, p, j, t] = e4m3(XS * x[b*T+t, (2g+j)*128+p])
        x8_d = nc.dram_tensor("x8", [NBLK, NG, 128, 2, T], F8,
                              kind="ExternalInput")
        xk8_d = nc.dram_tensor("xk8", [NBLK, NG, 128, 2, TB8], F8,
                               kind="ExternalInput")
        w8_d = {
            n: nc.dram_tensor(n, [NG, 128, 2, E], F8, kind="ExternalInput")
            for n in ("wq8", "wk8")
        }
    else:
        x_d = nc.dram_tensor("x", [NBLK, NCH, 128, T], F16,
                             kind="ExternalInput")
    xkv_d = nc.dram_tensor("xkv", [NBLK, NCH, 128, TBMAX], F16,
                           kind="ExternalInput")
    vones_d = nc.dram_tensor("vones", [128, CLOC], F16, kind="ExternalInput")
    ident_d = nc.dram_tensor("ident", [128, 128], F16, kind="ExternalInput")
    # fp16 weights host-prearranged per 128-row chunk
    w16_names = ("wv", "wo") if fp8 else ("wq", "wk", "wv", "wo")
    w_d = {
        n: nc.dram_tensor(n, [NCH, 128, E], F16, kind="ExternalInput")
        for n in w16_names
    }
    if with_bias:
        bqkv_d = {
            n: nc.dram_tensor(n, [128, NCH], F32, kind="ExternalInput")
            for n in ("bq", "bk", "bv")
        }
        bo_d = nc.dram_tensor("bo", [1, E], F16, kind="ExternalInput")
    o_d = nc.dram_tensor("o", [CLOC, R, E], F16, kind="ExternalOutput")

    with tile.TileContext(nc) as tc:
        with (
            tc.tile_pool(name="const", bufs=1) as const,
            tc.tile_pool(name="blk", bufs=2) as blkp,
            tc.tile_pool(name="col", bufs=4) as colp,
            tc.tile_pool(name="psmm", bufs=2, space="PSUM") as psmm,
            tc.tile_pool(name="pss", bufs=2, space="PSUM") as pssp,
            tc.tile_pool(name="pscx", bufs=2, space="PSUM") as pscx,
            tc.tile_pool(name="pstr", bufs=2, space="PSUM") as pstr,
        ):
            # ---- constants / weights
            w_sb = {
                n: [const.tile([128, E], F16, tag=f"{n}{k}", name=f"w_{n}{k}")
                    for k in range(NCH)]
                for n in w16_names
            }
            if fp8:
                w8_sb = {
                    n: [const.tile([128, 2, E], F8, tag=f"{n}{g}",
                                   name=f"w_{n}{g}")
                        for g in range(NG)]
                    for n in ("wq8", "wk8")
                }
            vones_sb = const.tile([128, CLOC], F16, tag="vones")
            ident_sb = const.tile([128, 128], F16, tag="ident")
            if fp8:
                x8t0 = [blkp.tile([128, 2, T], F8, tag=f"x8{g}",
                                  name=f"x8t0_{g}", bufs=3)
                        for g in range(NG)]
                xk8t0 = [blkp.tile([128, 2, TB8], F8, tag=f"xk8{g}",
                                   name=f"xk8t0_{g}", bufs=3)
                         for g in range(NG)]
            else:
                xt0 = [blkp.tile([128, T], F16, tag=f"xt{k}", name=f"xt0_{k}",
                                 bufs=3)
                       for k in range(NCH)]
            xk0 = [blkp.tile([128, TBMAX], F16, tag=f"xk{k}", name=f"xk0_{k}",
                             bufs=3)
                   for k in range(NCH)]
            qs = (nc.sync, nc.scalar, nc.gpsimd)
            # ident FIRST on the fast sync queue: it feeds the HAM warm-up
            # burst, which must start well before the bulk startup DMAs land
            nc.sync.dma_start(ident_sb[:], ident_d.ap())
            nc.gpsimd.dma_start(vones_sb[:], vones_d.ap())
            # stripe the Q-phase inputs first, then K, V, O phases in
            # consumption order so block-0 computes as chunks land
            if fp8:
                for g in range(NG):
                    qs[(2 * g) % 3].dma_start(x8t0[g][:], x8_d.ap()[0, g])
                    qs[(2 * g + 1) % 3].dma_start(
                        w8_sb["wq8"][g][:], w8_d["wq8"].ap()[g]
                    )
                for g in range(NG):
                    qs[(2 * g) % 3].dma_start(
                        xk8t0[g][:, :, 0:tb[0]],
                        xk8_d.ap()[0, g, :, :, 0:tb[0]],
                    )
                    qs[(2 * g + 1) % 3].dma_start(
                        w8_sb["wk8"][g][:], w8_d["wk8"].ap()[g]
                    )
            else:
                for k in range(NCH):
                    qs[(2 * k) % 3].dma_start(out=xt0[k][:], in_=x_d.ap()[0, k])
                    qs[(2 * k + 1) % 3].dma_start(
                        w_sb["wq"][k][:], w_d["wq"].ap()[k]
                    )
                for k in range(NCH):
                    qs[(2 * k) % 3].dma_start(
                        out=xk0[k][:, 0:tb[0]], in_=xkv_d.ap()[0, k, :, 0:tb[0]]
                    )
                    qs[(2 * k + 1) % 3].dma_start(
                        w_sb["wk"][k][:], w_d["wk"].ap()[k]
                    )
            if fp8:
                # fp16 packed keys still feed the V projection; they are
                # needed (t+~18us) before the wv weights finish, so issue
                # all six ahead of wv
                for k in range(NCH):
                    qs[k % 3].dma_start(
                        out=xk0[k][:, 0:tb[0]], in_=xkv_d.ap()[0, k, :, 0:tb[0]]
                    )
            for k in range(NCH):
                qs[(2 * k + 1) % 3].dma_start(w_sb["wv"][k][:], w_d["wv"].ap()[k])
            for k in range(NCH):
                qs[(2 * k + 1) % 3].dma_start(w_sb["wo"][k][:], w_d["wo"].ap()[k])
            if with_bias:
                bqkv_sb = {}
                for n in ("bq", "bk", "bv"):
                    bqkv_sb[n] = const.tile([128, NCH], F32, tag=n, name=f"b_{n}")
                    nc.gpsimd.dma_start(bqkv_sb[n][:], bqkv_d[n].ap())
                bo_sb = const.tile([1, E], F16, tag="bo")
                nc.gpsimd.dma_start(bo_sb[:], bo_d.ap())
                ones_sb = const.tile([1, 128], F16, tag="ones")
                nc.gpsimd.memset(ones_sb[:], 1.0)

            # ---- HAM warm-up: ~2.9us of back-to-back dummy matmuls right
            # after the (tiny, first-issued) ident DMA lands.  The PE clock
            # gate needs one busy 4096-cycle window to go 1.2 -> 2.4 GHz;
            # without this the whole DMA-paced block-0 projection runs cold
            # (the gate only flipped at ~18us into the baseline kernel).
            warm = psmm.tile([128, T], F32, tag="mm", name="warm")
            for _ in range(27):
                nc.tensor.matmul(
                    warm[:, 0:128], ident_sb[:], ident_sb[:],
                    start=True, stop=True,
                )

            pending_wo = None
            for b in range(NBLK):
                TB = tb[b]
                loff = [off[b * BLK + t] - off[b * BLK] for t in range(BLK)]

                if b == 0:
                    xk = xk0
                    if fp8:
                        x8t, xk8t = x8t0, xk8t0
                    else:
                        xt = xt0
                else:
                    # steady state: all input DMAs on the sync queue so the
                    # scalar queue is free for PSUM-drain copies (the psmm
                    # ring stalls projections if the drain lags)
                    xk = [blkp.tile([128, TBMAX], F16, tag=f"xk{k}",
                                    name=f"xk_{k}", bufs=3)
                          for k in range(NCH)]
                    if fp8:
                        x8t = [blkp.tile([128, 2, T], F8, tag=f"x8{g}",
                                         name=f"x8_{g}", bufs=3)
                               for g in range(NG)]
                        xk8t = [blkp.tile([128, 2, TB8], F8, tag=f"xk8{g}",
                                          name=f"xk8_{g}", bufs=3)
                                for g in range(NG)]
                        for g in range(NG):
                            nc.sync.dma_start(x8t[g][:], x8_d.ap()[b, g])
                        for g in range(NG):
                            nc.sync.dma_start(
                                xk8t[g][:, :, 0:TB],
                                xk8_d.ap()[b, g, :, :, 0:TB],
                            )
                    else:
                        xt = [blkp.tile([128, T], F16, tag=f"xt{k}",
                                        name=f"xt_{k}", bufs=3)
                              for k in range(NCH)]
                        for ec in range(NCH):
                            nc.sync.dma_start(out=xt[ec][:], in_=x_d.ap()[b, ec])
                    for ec in range(NCH):
                        nc.sync.dma_start(
                            out=xk[ec][:, 0:TB],
                            in_=xkv_d.ap()[b, ec, :, 0:TB],
                        )

                # ---- Q^T over all tokens; K^T, V^T over packed keys ----
                qt = blkp.tile([128, NCH, T], F16, tag="qt")
                kt = blkp.tile([128, NCH, TBMAX], F16, tag="kt")
                vt = blkp.tile([128, NCH, TBMAX], F16, tag="vt")
                # zero-padded Q buffer (used below); memset early so the
                # gpsimd engine does it during the projection matmuls
                qz = blkp.tile([128, H, T], F16, tag="qz")
                if b < 2:
                    nc.gpsimd.memset(qz[:], 0.0)  # pool ring: zeros persist

                def drain_proj(dst, bname, co, ps, n_):
                    if with_bias:
                        nc.scalar.activation(
                            dst[:, co, 0:n_], ps[:, 0:n_], Act.Identity,
                            bias=bqkv_sb[bname][:, co : co + 1],
                        )
                    elif co % 2 == 0:   # alternate drain engines so the
                        nc.scalar.copy(dst[:, co, 0:n_], ps[:, 0:n_])
                    else:               # psum rings free promptly
                        nc.vector.tensor_copy(dst[:, co, 0:n_], ps[:, 0:n_])

                # projection emitters: (weight-list, pair-mode, src, n, dst)
                if fp8:
                    projs = (
                        (w8_sb["wq8"], True, x8t, T, qt, "bq"),
                        (w8_sb["wk8"], True, xk8t, TB, kt, "bk"),
                        (w_sb["wv"], False, xk, TB, vt, "bv"),
                    )
                else:
                    projs = (
                        (w_sb["wq"], False, xt, T, qt, "bq"),
                        (w_sb["wk"], False, xk, TB, kt, "bk"),
                        (w_sb["wv"], False, xk, TB, vt, "bv"),
                    )

                def proj_mm(ps, wtiles, pair, src_, n_, k, nk, co):
                    if pair:
                        nc.tensor.matmul(
                            ps[:, 0:n_],
                            wtiles[k][:, :, co * 128 : (co + 1) * 128],
                            src_[k][:, :, 0:n_],
                            start=(k == 0), stop=(k == nk - 1),
                            perf_mode=DR,
                        )
                    else:
                        nc.tensor.matmul(
                            ps[:, 0:n_],
                            wtiles[k][:, co * 128 : (co + 1) * 128],
                            src_[k][:, 0:n_],
                            start=(k == 0), stop=(k == nk - 1),
                        )

                for wtiles, pair, src_, n_, dst, bname in projs:
                    nk = NG if pair else NCH
                    if b == 0:
                        # k-outer with 6 concurrent psum groups: the first
                        # matmul only needs group 0 of the weights and x, so
                        # the PE starts as soon as the first chunks land.
                        pss6 = [
                            psmm.tile([128, T], F32, tag="mm", name="p6a"),
                            psmm.tile([128, T], F32, tag="mm", name="p6b"),
                            pssp.tile([128, 512], F32, tag="s", name="p6c"),
                            pssp.tile([128, 512], F32, tag="s", name="p6d"),
                            pscx.tile([128, 512], F32, tag="cx", name="p6e"),
                            pscx.tile([128, 512], F32, tag="cx", name="p6f"),
                        ]
                        for k in range(nk):
                            for co in range(NCH):
                                proj_mm(pss6[co], wtiles, pair, src_, n_,
                                        k, nk, co)
                        for co in range(NCH):
                            drain_proj(dst, bname, co, pss6[co], n_)
                    else:
                        for co in range(NCH):
                            # alternate psum pools: pssp is idle during the
                            # projection phase, and a 2-deep ring stalls the
                            # PE ~566ns/block waiting on drain turnaround
                            if co % 2 == 0:
                                ps = psmm.tile([128, T], F32, tag="mm",
                                               name="ps")
                            else:
                                ps = pssp.tile([128, 512], F32, tag="s",
                                               name="ps2")
                            for k in range(nk):
                                proj_mm(ps, wtiles, pair, src_, n_, k, nk, co)
                            drain_proj(dst, bname, co, ps, n_)

                # ---- per-head zero-padded Q^T (base-partition-64 matmuls
                # into shared PSUM banks crash HW; contract K=128 instead,
                # with the other head's rows zeroed on the Q side.  Mixing
                # 64- and 128-row matmuls stalls the PE array ~100ns per
                # reconfig, so all heads use the K=128 form.)  Two fused 3D
                # copies: even heads sit at partitions 0-63 of their chunk,
                # odd heads at 64-127.
                qzv = qz.rearrange("p (c two) t -> p c two t", two=2)
                nc.vector.tensor_copy(qzv[0:64, :, 0, :], qt[0:64, :, :])
                nc.vector.tensor_copy(qzv[64:128, :, 1, :], qt[64:128, :, :])

                va = blkp.tile([128, BLK, H * 65], F16, tag="va")
                ets, pscs, ctxnts = {}, {}, {}

                def emit_vtrans(t):
                    # V natural per column: PE transpose of packed V^T,
                    # then DVE assembly into 65-stride head slots + ones.
                    p = b * BLK + t
                    kc = kcap[p]
                    lo = loff[t]
                    vtp = pstr.tile([128, NCH, 128], F16, tag="tr", name="vtp")
                    for ec in range(NCH):
                        nc.tensor.transpose(
                            vtp[0:kc, ec, :],
                            vt[:, ec, lo : lo + kc],
                            ident_sb[:],
                        )
                    dst = va[0:kc, t, :].rearrange(
                        "p (h x) -> p h x", x=65
                    )[:, :, 0:64]
                    src = vtp[0:kc, :, :].rearrange(
                        "p c (h d) -> p (c h) d", d=64
                    )
                    if with_bias:
                        # zero padding rows (v=bv there otherwise)
                        nc.vector.tensor_mul(
                            dst, src,
                            vones_sb[0:kc, p : p + 1]
                            .unsqueeze(2)
                            .broadcast_to((kc, H, 64)),
                        )
                    else:
                        nc.vector.tensor_copy(dst, src)
                    ones_dst = va[0:kc, t, :].rearrange(
                        "p (h x) -> p h x", x=65
                    )[:, :, 64:65]
                    nc.vector.tensor_copy(
                        ones_dst,
                        vones_sb[0:kc, p : p + 1].unsqueeze(2).broadcast_to(
                            (kc, H, 1)
                        ),
                    )

                def emit_scores(t):
                    p = b * BLK + t
                    kc = kcap[p]
                    lo = loff[t]
                    et = colp.tile([128, H * 128], F16, tag="et",
                                   name=f"et_{b}_{t}")
                    for g3 in range(3):
                        pss = pssp.tile([128, 512], F32, tag="s", name="pss")
                        for hh in range(4):
                            h = g3 * 4 + hh
                            nc.tensor.matmul(
                                pss[0:kc, hh * 128 : (hh + 1) * 128],
                                kt[:, h // 2, lo : lo + kc],
                                qz[:, h, t * 128 : (t + 1) * 128],
                                start=(hh == 0),
                                stop=(hh == 3),
                            )
                        # fp8 path: q,k carry the 2^21 quantization scale;
                        # exp divides it back out for free
                        nc.scalar.activation(
                            et[0:kc, g3 * 512 : (g3 + 1) * 512],
                            pss[0:kc, :], Act.Exp,
                            scale=EXPSCALE if fp8 else 1.0,
                        )
                    ets[t] = et

                def emit_pv_wo(t, wo_cg=None, wo_ctxnt=None, cross_block=False):
                    # PV of column t interleaved 1:1 with the previous
                    # column's output projection: the Wo matmuls (no fresh
                    # dependencies) absorb PV's exp-wait instead of the PE
                    # FIFO head-of-line blocking on it, and PV's LDWEIGHTS
                    # hide under Wo's longer matmuls.
                    p = b * BLK + t
                    kc = kcap[p]
                    et = ets.pop(t)
                    osb = None
                    if wo_ctxnt is not None:
                        osb = colp.tile([128, E], F16, tag="osb", name="osb")
                    psc = []
                    for g2 in range(2):
                        pc = pscx.tile([128, 390], F32, tag="cx", name="pc")
                        po = None
                        if wo_ctxnt is not None:
                            po = psmm.tile([128, T], F32, tag="mm", name="po")
                            if with_bias:
                                nc.tensor.matmul(
                                    po[:, 0:384], ones_sb[:],
                                    bo_sb[:, g2 * 384 : (g2 + 1) * 384],
                                    start=True, stop=False,
                                )
                        # group-granularity interleave: the whole Wo half
                        # runs while PV's first matmul waits on exp.  (A
                        # finer 1:1 MM interleave of the two open groups --
                        # alternating kc-row and 128-row stationaries --
                        # crashed the device: NRT_EXEC_UNIT_UNRECOVERABLE.)
                        if po is not None:
                            for i in range(6):
                                nc.tensor.matmul(
                                    po[:, 0:384],
                                    wo_ctxnt[:, i, :],
                                    w_sb["wo"][i][:, g2 * 384 : (g2 + 1) * 384],
                                    start=(i == 0 and not with_bias),
                                    stop=(i == 5),
                                )
                        for i in range(6):
                            h = g2 * 6 + i
                            nc.tensor.matmul(
                                pc[:, i * 65 : (i + 1) * 65],
                                et[0:kc, h * 128 : (h + 1) * 128],
                                va[0:kc, t, h * 65 : (h + 1) * 65],
                                start=(i == 0),
                                stop=(i == 5),
                            )
                        if po is not None:
                            if cross_block:  # vector congested at block start
                                nc.scalar.copy(
                                    osb[:, g2 * 384 : (g2 + 1) * 384],
                                    po[:, 0:384],
                                )
                            else:
                                nc.vector.tensor_copy(
                                    osb[:, g2 * 384 : (g2 + 1) * 384],
                                    po[:, 0:384],
                                )
                        psc.append(pc)
                    if wo_ctxnt is not None:
                        if wo_cg == CLOC - 1:
                            oeng = nc.scalar
                        elif wo_cg == CLOC - 2:
                            oeng = nc.sync
                        else:
                            oeng = nc.gpsimd
                        oeng.dma_start(o_d.ap()[wo_cg], osb[:])
                    pscs[t] = psc

                def emit_norm_tr(t):
                    psc = pscs.pop(t)
                    recip = colp.tile([128, H], F32, tag="recip", name="recip")
                    ctxn = colp.tile([128, E], F16, tag="ctxn", name="ctxn")
                    for g2 in range(2):
                        grp = psc[g2].rearrange("p (h x) -> p h x", x=65)
                        nc.vector.reciprocal(
                            recip[:, g2 * 6 : (g2 + 1) * 6].unsqueeze(2),
                            grp[:, :, 64:65],
                        )
                        nc.vector.tensor_mul(
                            ctxn[:, g2 * 384 : (g2 + 1) * 384].rearrange(
                                "p (h d) -> p h d", d=64
                            ),
                            grp[:, :, 0:64],
                            recip[:, g2 * 6 : (g2 + 1) * 6]
                            .unsqueeze(2)
                            .broadcast_to((128, 6, 64)),
                        )
                    pst = pstr.tile([128, NCH, 128], F16, tag="tr", name="pst")
                    for ec in range(NCH):
                        nc.tensor.transpose(
                            pst[:, ec, :],
                            ctxn[:, ec * 128 : (ec + 1) * 128],
                            ident_sb[:],
                        )
                    ctxnt = colp.tile([128, NCH, 128], F16, tag="ctxnt",
                                      name="ctxnt")
                    nc.vector.tensor_copy(ctxnt[:], pst[:])
                    ctxnts[t] = ctxnt

                def emit_wo_solo(t, cg):
                    # last column of the final block: nothing left to
                    # interleave with
                    ctxnt = ctxnts.pop(t)
                    osb = colp.tile([128, E], F16, tag="osb", name="osb")
                    for half in range(2):
                        po = psmm.tile([128, T], F32, tag="mm", name="po")
                        if with_bias:
                            nc.tensor.matmul(
                                po[:, 0:384], ones_sb[:],
                                bo_sb[:, half * 384 : (half + 1) * 384],
                                start=True, stop=False,
                            )
                        for k in range(NCH):
                            nc.tensor.matmul(
                                po[:, 0:384],
                                ctxnt[:, k, :],
                                w_sb["wo"][k][:, half * 384 : (half + 1) * 384],
                                start=(k == 0 and not with_bias),
                                stop=(k == NCH - 1),
                            )
                        nc.vector.tensor_copy(
                            osb[:, half * 384 : (half + 1) * 384],
                            po[:, 0:384],
                        )
                    nc.scalar.dma_start(o_d.ap()[cg], osb[:])

                last_blk = b == NBLK - 1
                emit_scores(0)
                emit_vtrans(0)
                emit_scores(1)
                for t in range(BLK):
                    if t >= 1:
                        emit_pv_wo(t, b * BLK + t - 1, ctxnts.pop(t - 1))
                    elif pending_wo is not None:
                        pcg, pctx = pending_wo   # last column of prev block
                        emit_pv_wo(t, pcg, pctx, cross_block=True)
                        pending_wo = None
                    else:
                        emit_pv_wo(t)
                    if t + 1 < BLK:
                        emit_vtrans(t + 1)
                    if t + 2 < BLK:
                        emit_scores(t + 2)
                    emit_norm_tr(t)
                if last_blk:
                    emit_wo_solo(BLK - 1, b * BLK + BLK - 1)
                else:
                    pending_wo = (b * BLK + BLK - 1, ctxnts.pop(BLK - 1))
    nc.compile()
    return nc


_PROGRAMS = {}


def _get_program(with_bias: bool, kcap: tuple):
    key = (with_bias, kcap)
    if key not in _PROGRAMS:
        _PROGRAMS[key] = build_program(with_bias, kcap)
    return _PROGRAMS[key]


def plan_columns(mask):
    """mask: (R, C) bool, True = valid key.  Returns per-core column lists,
    per-position capacities, and per-(core,position) key indices."""
    counts = mask.sum(axis=0)                       # (C,)
    order = np.argsort(-counts, kind="stable")
    cols = [[int(order[8 * p + i]) for p in range(CLOC)] for i in range(NCORES)]
    kcap = tuple(
        int(counts[order[8 * p : 8 * p + 8]].max()) for p in range(CLOC)
    )
    return cols, kcap


def make_in_maps(x, self_attn_padding_mask, Wq, bq, Wk, bk, Wv, bv, Wo, bo,
                 with_bias, cols, kcap):
    scaling = float(D) ** -0.5
    fp8 = not with_bias
    def prep_w(W, s=1.0):
        w = (np.asarray(W, np.float32) * s).astype(np.float16)
        return np.ascontiguousarray(w.reshape(NCH, 128, E))
    def prep_w8(W, s):
        # [NG, 128, 2, E]: row (2g+j)*128+p at [g, p, j]
        w = (np.asarray(W, np.float32) * s).astype(E4)
        return np.ascontiguousarray(
            w.reshape(NG, 2, 128, E).transpose(0, 2, 1, 3)
        )
    wv, wo = prep_w(Wv), prep_w(Wo)
    if fp8:
        wq8 = prep_w8(Wq, scaling * WSQ)
        wk8 = prep_w8(Wk, WSK)
    else:
        wq, wk = prep_w(Wq, scaling), prep_w(Wk)
    mask = np.asarray(self_attn_padding_mask)[0]                   # (R, C)
    x32 = np.asarray(x, np.float32)[:, :, 0, :]                    # (R, C, E)
    xf = x32.astype(np.float16)
    ident = np.eye(128, dtype=np.float16)
    off = np.concatenate([[0], np.cumsum(kcap)]).astype(int)
    tb = [int(off[(b + 1) * BLK] - off[b * BLK]) for b in range(NBLK)]
    TBMAX = max(tb)
    TB8 = (TBMAX + 15) // 16 * 16
    in_maps = []
    for i in range(NCORES):
        ci = cols[i]
        # packed keys (fp16 for V, fp8 for K), per-block padded
        xps = np.zeros((NBLK, NCH, 128, TBMAX), np.float16)
        xp32b = np.zeros((NBLK, TB8, E), np.float32)
        vo = np.zeros((128, CLOC), np.float16)
        for b in range(NBLK):
            xp = np.zeros((TBMAX, E), np.float16)
            for t in range(BLK):
                p = b * BLK + t
                col = ci[p]
                idx = np.nonzero(mask[:, col])[0]
                lo = int(off[p] - off[b * BLK])
                xp[lo : lo + len(idx)] = xf[idx, col]
                xp32b[b, lo : lo + len(idx)] = x32[idx, col]
                vo[: len(idx), p] = 1.0
            xps[b] = xp.reshape(TBMAX, NCH, 128).transpose(1, 2, 0)
        m = {
            "xkv": xps,
            "vones": vo,
            "wv": wv, "wo": wo,
            "ident": ident,
        }
        if fp8:
            # full x^T in DoubleRow pair layout [NBLK, NG, 128, 2, T]
            x8 = (
                (x32[:, ci] * XS).astype(E4)       # (R, CLOC, E)
                .transpose(1, 0, 2)                # (CLOC, R, E)
                .reshape(NBLK, T, NG, 2, 128)
                .transpose(0, 2, 4, 3, 1)          # (NBLK, NG, 128, 2, T)
            )
            xk8 = (
                (xp32b * XS).astype(E4)            # (NBLK, TB8, E)
                .reshape(NBLK, TB8, NG, 2, 128)
                .transpose(0, 2, 4, 3, 1)          # (NBLK, NG, 128, 2, TB8)
            )
            m["x8"] = np.ascontiguousarray(x8)
            m["xk8"] = np.ascontiguousarray(xk8)
            m["wq8"] = wq8
            m["wk8"] = wk8
        else:
            xs = (
                xf[:, ci]                          # (R, CLOC, E)
                .transpose(1, 0, 2)                # (CLOC, R, E)
                .reshape(NBLK, T, NCH, 128)
                .transpose(0, 2, 3, 1)             # (NBLK, NCH, 128, T)
            )
            m["x"] = np.ascontiguousarray(xs)
            m["wq"] = wq
            m["wk"] = wk
        if with_bias:
            m["bq"] = np.ascontiguousarray(
                (np.asarray(bq, np.float32) * scaling).reshape(NCH, 128).T
            )
            m["bk"] = np.ascontiguousarray(
                np.asarray(bk, np.float32).reshape(NCH, 128).T
            )
            m["bv"] = np.ascontiguousarray(
                np.asarray(bv, np.float32).reshape(NCH, 128).T
            )
            m["bo"] = np.asarray(bo, np.float32).astype(np.float16).reshape(1, E)
        in_maps.append(m)
    return in_maps


def assemble_output(shards, cols):
    out = np.empty((R, C, 1, E), np.float32)
    for i in range(NCORES):
        out[:, cols[i], 0, :] = np.asarray(shards[i], np.float32).transpose(1, 0, 2)
    return out


def kernel(x, self_attn_padding_mask, Wq, bq, Wk, bk, Wv, bv, Wo, bo):
    global LAST_RESULTS
    with_bias = any(
        bool(np.any(np.asarray(b))) for b in (bq, bk, bv, bo)
    )
    mask = np.asarray(self_attn_padding_mask)[0]
    cols, kcap = plan_columns(mask)
    nc = _get_program(with_bias, kcap)
    in_maps = make_in_maps(
        x, self_attn_padding_mask, Wq, bq, Wk, bk, Wv, bv, Wo, bo, with_bias,
        cols, kcap,
    )
    trace = os.environ.get("KERNEL_TRACE", "") not in ("", "0")
    res = run_bass_kernel_spmd(
        nc, in_maps, core_ids=list(range(NCORES)), trace=trace
    )
    LAST_RESULTS = res
    return assemble_output([res.results[i]["o"] for i in range(NCORES)], cols)
